# revision 1
# baseline (speedup 1.0000x reference)
"""Distributed DMPNN (2-layer GRU message passing) for 8 TRN2 NeuronCores.

Math identity exploited (linearity of segment_sum):
    msg  = concat(x[src], ea) @ Wm^T = y[src] + ea @ We^T,  y = x @ Wx^T
    agg  = seg_sum(msg, dst) = seg_sum(y[src], dst) + A @ We^T,
           A = seg_sum(ea, dst)  (layer independent)

Sharding: edges sorted by dst; core c owns dst in [c*NPC, (c+1)*NPC).
Each core computes agg and the GRU update for its own node range only.
The single collective is an AllGather of y2 = x1 @ Wx2^T between layers.

Per-core device pipeline:
  P1: y1 = x @ Wx1^T for ALL nodes (replicated; x^T is a per-core input)
      -> y1_dram (bf16 rows, 256B each)
  P2: per dst-tile t (128 dst nodes):
      - dma_gather y1 rows for the tile's padded edge slots
      - build one-hot(dst) via iota==dstf (DVE), bf16
      - PE scatter matmuls: psum[dst,0:128] += onehot^T @ y1_gath
                            psum[dst,128:192] += onehot^T @ ea
      - A^T tile via PE transpose; + A@We1^T into psum[:,0:128]
      - GRU1 matmuls + elementwise -> x1 (node major) + x1^T (bf16)
      - y2 tile = x1 @ Wx2^T -> y2_local dram
  P3: AllGather y2_local -> y2_full
  P4: per dst-tile t: gather y2, scatter (aggT orientation), + We2-term
      via A^T, GRU2 -> out rows.
"""

import numpy as np

import concourse.bass as bass
import concourse.mybir as mybir
import concourse.bacc as bacc
import concourse.tile as tile
from concourse.masks import make_identity

F32 = mybir.dt.float32
BF16 = mybir.dt.float16  # 16-bit compute dtype (fp16: more mantissa than bf16)
I16 = mybir.dt.int16
NPBF16 = np.dtype(mybir.dt.np(BF16))

N_CORES = 8
P = 128


# ---------------------------------------------------------------- host side
def preprocess(x, edge_index, edge_attr,
               W_msg1, Wih1, Whh1, bih1, bhh1,
               W_msg2, Wih2, Whh2, bih2, bhh2, force_k_chunks=None):
    """Host-side integer/layout preprocessing. Returns (meta, in_maps)."""
    x = np.asarray(x, np.float32)
    n_nodes, hidden = x.shape
    edge_dim = edge_attr.shape[1]
    assert hidden == 128 and edge_dim == 64
    assert n_nodes % N_CORES == 0
    npc = n_nodes // N_CORES          # nodes per core
    nt = (npc + P - 1) // P           # dst tiles per core
    nta = (n_nodes + P - 1) // P      # tiles over all nodes

    src = np.asarray(edge_index[0], np.int64)
    dst = np.asarray(edge_index[1], np.int64)
    ea = np.asarray(edge_attr, np.float32)

    # Balance the kernel's per-core 64-node windows: round-robin nodes by
    # in-degree so window edge counts are near-uniform (shrinks the uniform
    # chunk padding and balances cores). new_of_old: old id -> new position.
    W0 = 64
    npc0 = n_nodes // N_CORES
    wpc = (npc0 + W0 - 1) // W0          # windows per core (last may be short)
    caps, bases = [], []
    for c in range(N_CORES):
        for wi in range(wpc):
            cap = min(W0, npc0 - wi * W0)
            caps.append(cap)
            bases.append(c * npc0 + wi * W0)
    caps = np.asarray(caps); bases = np.asarray(bases)
    deg = np.bincount(dst, minlength=n_nodes)
    by_deg = np.argsort(-deg, kind="stable")
    full = np.nonzero(caps == W0)[0]
    short = np.nonzero(caps < W0)[0]
    n_short_nodes = int(caps[short].sum())
    new_of_old = np.empty(n_nodes, np.int64)
    # HIGHEST-degree nodes fill the short windows (pulls the full-window mean
    # down, enabling a smaller uniform chunk count)
    k = 0
    for wnd in short:
        for slot in range(caps[wnd]):
            new_of_old[by_deg[k]] = bases[wnd] + slot
            k += 1
    # serpentine round-robin over the full windows for the rest
    nf = len(full)
    rest = by_deg[n_short_nodes:]
    widx = np.concatenate([full, full[::-1]])
    slot_in_w = np.zeros(len(caps), np.int64)
    member = [[] for _ in range(len(caps))]
    load = np.zeros(len(caps), np.int64)
    for i, node in enumerate(rest):
        wnd = widx[i % (2 * nf)]
        member[wnd].append(node)
        load[wnd] += deg[node]
        slot_in_w[wnd] += 1
    assert (slot_in_w[full] == W0).all()
    # local search: swap nodes between max- and min-load windows until every
    # full window fits the target chunk budget (or no progress)
    target = ((int(load[full].mean()) // P) + 1) * P
    for _ in range(4000):
        fl = load[full]
        hi = full[int(np.argmax(fl))]
        if load[hi] <= target:
            break
        lo = full[int(np.argmin(fl))]
        mh, ml = member[hi], member[lo]
        dh = np.array([deg[n] for n in mh]); dl = np.array([deg[n] for n in ml])
        need = load[hi] - target
        diff = dh[:, None] - dl[None, :]
        ok = diff > 0
        if not ok.any():
            break
        cand = np.where(ok, diff, 10**9)
        i_, j_ = np.unravel_index(int(np.argmin(np.abs(cand - need))),
                                  cand.shape)
        d = int(diff[i_, j_])
        if d <= 0:
            break
        mh[i_], ml[j_] = ml[j_], mh[i_]
        load[hi] -= d; load[lo] += d
    for wnd in full:
        for slot, node in enumerate(member[wnd]):
            new_of_old[node] = bases[wnd] + slot
    old_of_new = np.empty(n_nodes, np.int64)
    old_of_new[new_of_old] = np.arange(n_nodes)

    x = x[old_of_new]                 # x in NEW node order
    src = new_of_old[src]
    dst = new_of_old[dst]

    order = np.lexsort((src, dst))
    src_s, dst_s = src[order], dst[order]
    ea_s = ea[order]

    # per (core, tile, 64-window) edge counts -> uniform KW chunks per window
    W = 64
    WT = P // W                       # windows per 128-dst tile
    core_of = dst_s // npc
    dloc = dst_s - core_of * npc
    w_of = dloc // W                  # window within core (0 .. nt*WT-1)
    counts = np.zeros((N_CORES, nt * WT), np.int64)
    np.add.at(counts, (core_of, w_of), 1)
    kw = int(np.max((counts + P - 1) // P))
    if force_k_chunks is not None:
        assert force_k_chunks >= kw * WT
        kw = force_k_chunks // WT
    k_chunks = kw * WT
    slots_per_tile = k_chunks * P
    slots = nt * slots_per_tile

    meta = dict(n_nodes=n_nodes, npc=npc, nt=nt, nta=nta, k_chunks=k_chunks,
                kw=kw, slots=slots, new_of_old=new_of_old)

    # slot assignment per core
    in_maps = []
    # boundaries of (core, window) groups in the sorted edge array
    grp = core_of * (nt * WT) + w_of
    starts = np.searchsorted(grp, np.arange(N_CORES * nt * WT), side="left")
    ends = np.searchsorted(grp, np.arange(N_CORES * nt * WT), side="right")

    bz = not (np.any(bih1) or np.any(bhh1) or np.any(bih2) or np.any(bhh2))
    meta["biases_zero"] = bz
    assert bz, "nonzero biases not implemented in kernel yet"

    for c in range(N_CORES):
        g_idx = np.zeros(slots, np.int64)
        dst_f = np.full(slots, -1.0, np.float32)
        ea_arr = np.zeros((slots, edge_dim), np.float32)
        for wg in range(nt * WT):
            g = c * (nt * WT) + wg
            s0, s1 = starts[g], ends[g]
            cnt = s1 - s0
            base = wg * kw * P
            g_idx[base:base + cnt] = src_s[s0:s1]
            dst_f[base:base + cnt] = (dloc[s0:s1] - wg * W).astype(np.float32)
            ea_arr[base:base + cnt] = ea_s[s0:s1]

        # wrapped int16 gather indices, per tile: [16, k_chunks*8] blocks,
        # replicated to all 8 Q7-core partition stripes -> [128, slots//16]
        gidx_w = np.zeros((16, slots // 16), np.int16)
        for t in range(nt):
            blk = g_idx[t * slots_per_tile:(t + 1) * slots_per_tile]
            gidx_w[:, t * (slots_per_tile // 16):(t + 1) * (slots_per_tile // 16)] = \
                blk.reshape(-1, 16).T.astype(np.int16)
        gidx_w = np.tile(gidx_w, (8, 1))

        # dstf [128, nt*k_chunks] (partition = lane within chunk)
        dstf = dst_f.reshape(nt * k_chunks, P).T.astype(NPBF16).copy()
        # ea arranged [128, nt*k_chunks*64]
        ea_in = np.ascontiguousarray(
            ea_arr.reshape(nt * k_chunks, P, edge_dim).transpose(1, 0, 2)
        ).reshape(P, nt * k_chunks * edge_dim).astype(NPBF16)

        # node-major x shard (f32) [128, nt*128]
        xs = np.zeros((P, nt * P), np.float32)
        rows = x[c * npc:(c + 1) * npc]  # [npc, 128]
        xsr = np.zeros((nt * P, P), np.float32)
        xsr[:npc] = rows
        xs = np.ascontiguousarray(
            xsr.reshape(nt, P, P).transpose(1, 0, 2)).reshape(P, nt * P)
        # transposed x shard (bf16) [128, nt*128]
        xsT = np.zeros((P, nt * P), np.float32)
        xsT[:, :npc] = rows.T
        xsT = xsT.astype(NPBF16)

        in_maps.append({
            "xsT": xsT,
            "xs": xs.astype(np.float32),
            "gidx": gidx_w,
            "dstf": dstf,
            "ea": ea_in,
            "wx1r": np.ascontiguousarray(W_msg1[:, :128].T).astype(NPBF16),
            "we1r": np.ascontiguousarray(W_msg1[:, 128:].T).astype(NPBF16),
            "wih1t": np.ascontiguousarray(np.asarray(Wih1).T).astype(NPBF16),
            "whh1t": np.ascontiguousarray(np.asarray(Whh1).T).astype(NPBF16),
            "wx2r": np.ascontiguousarray(W_msg2[:, :128].T).astype(NPBF16),
            "we2r": np.ascontiguousarray(W_msg2[:, 128:].T).astype(NPBF16),
            "wih2t": np.ascontiguousarray(np.asarray(Wih2).T).astype(NPBF16),
            "whh2t": np.ascontiguousarray(np.asarray(Whh2).T).astype(NPBF16),
        })
    return meta, in_maps


# ---------------------------------------------------------------- device side
def build(meta, n_iters=1, single_core=False):
    n_nodes = meta["n_nodes"]
    npc, nt, nta, K = meta["npc"], meta["nt"], meta["nta"], meta["k_chunks"]
    KW = meta["kw"]
    W = 64
    spt = K * P  # slots per tile
    # one gather per dst tile when it fits a validated 4096-descriptor
    # packet budget; otherwise split at 2048 descriptors (also validated)
    GCH = 32 if K <= 32 else 16

    nc = bacc.Bacc("TRN2", target_bir_lowering=False, debug=False,
                   num_devices=1 if single_core else N_CORES)

    xsT_d = nc.dram_tensor("xsT", [P, nt * P], BF16, kind="ExternalInput")
    xs_d = nc.dram_tensor("xs", [P, nt * P], F32, kind="ExternalInput")
    gidx_d = nc.dram_tensor("gidx", [P, nt * spt // 16], I16, kind="ExternalInput")
    dstf_d = nc.dram_tensor("dstf", [P, nt * K], BF16, kind="ExternalInput")
    ea_d = nc.dram_tensor("ea", [P, nt * K * 64], BF16, kind="ExternalInput")
    w_d = {}
    for nm, shape in [("wx1r", [P, P]), ("we1r", [64, P]),
                      ("wih1t", [P, 384]), ("whh1t", [P, 384]),
                      ("wx2r", [P, P]), ("we2r", [64, P]),
                      ("wih2t", [P, 384]), ("whh2t", [P, 384])]:
        w_d[nm] = nc.dram_tensor(nm, shape, BF16, kind="ExternalInput")
    out_d = nc.dram_tensor("out", [npc, P], F32, kind="ExternalOutput")

    with tile.TileContext(nc) as tc:
        with tc.tile_pool(name="persist", bufs=1) as pp, \
             tc.tile_pool(name="work", bufs=3) as wp, \
             tc.tile_pool(name="small", bufs=3) as sp, \
             tc.tile_pool(name="psA", bufs=2, space="PSUM") as ppsA, \
             tc.tile_pool(name="psA2", bufs=1, space="PSUM") as ppsA2, \
             tc.tile_pool(name="psB", bufs=1, space="PSUM") as ppsB, \
             tc.tile_pool(name="psC", bufs=2, space="PSUM") as ppsC, \
             tc.tile_pool(name="psD", bufs=1, space="PSUM") as ppsD, \
             tc.tile_pool(name="dram", bufs=1, space="DRAM") as dp:

            # ---- persistent SBUF state
            xsT = pp.tile([P, nt * P], BF16, tag="xsT")
            nc.sync.dma_start(xsT[:], xsT_d[:])
            xs = pp.tile([P, nt * P], F32, tag="xs")
            nc.sync.dma_start(xs[:], xs_d[:])
            gidx = pp.tile([P, nt * spt // 16], I16, tag="gidx")
            nc.sync.dma_start(gidx[:], gidx_d[:])
            dstf = pp.tile([P, nt * K], BF16, tag="dstf")
            nc.sync.dma_start(dstf[:], dstf_d[:])
            w = {}
            for nm, h in w_d.items():
                w[nm] = pp.tile(list(h.shape), BF16, tag=nm, name=nm)
                nc.sync.dma_start(w[nm][:], h[:])

            ident = pp.tile([P, P], BF16, tag="ident")
            make_identity(nc, ident[:])
            iota_i = pp.tile([P, 1, W], I16, tag="iota_i")
            nc.gpsimd.iota(iota_i[:], pattern=[[0, 1], [1, W]], base=0,
                           channel_multiplier=0)
            iota_b = pp.tile([P, 1, W], BF16, tag="iota_b")
            nc.vector.tensor_copy(iota_b[:], iota_i[:])

            x1_nm = pp.tile([P, nt * P], F32, tag="x1_nm")
            x1T = pp.tile([P, nt * P], BF16, tag="x1T")
            AT = pp.tile([64, nt * P], BF16, tag="AT")

            for _ in range(n_iters):
                # ---- DRAM scratch (fresh per iteration: Shared tensors are
                # single-writer)
                y1_loc = dp.tile([npc, P], BF16, name="y1_loc")
                y1_dram = dp.tile([n_nodes, P], BF16, addr_space="Shared",
                                  name="y1_dram")
                y2_loc = dp.tile([npc, P], BF16, name="y2_loc")
                y2_full = dp.tile([n_nodes, P], BF16, addr_space="Shared",
                                  name="y2_full")
                # ============================ P1: y1 shard + AllGather
                for t in range(nt):
                    rows = min(P, npc - t * P)
                    psy = ppsC.tile([P, P], F32, tag="psy")
                    nc.tensor.matmul(psy[:], lhsT=xsT[:, t * P:(t + 1) * P],
                                     rhs=w["wx1r"][:], start=True, stop=True)
                    y1b = sp.tile([P, P], BF16, tag="y2b")
                    nc.vector.tensor_copy(y1b[:], psy[:])
                    nc.sync.dma_start(y1_loc[t * P:t * P + rows, :], y1b[0:rows, :])
                if single_core:
                    nc.sync.dma_start(y1_dram[0:npc, :], y1_loc[:])
                else:
                    nc.gpsimd.collective_compute(
                        "AllGather", mybir.AluOpType.bypass,
                        replica_groups=[list(range(N_CORES))],
                        ins=[y1_loc[:].opt()], outs=[y1_dram[:].opt()])

                # ============================ P2: layer 1 per dst tile
                for t in range(nt):
                    rows = min(P, npc - t * P)
                    yg = wp.tile([P, K, P], BF16, tag="yg")
                    for c0 in range(0, K, GCH):
                        c1 = min(c0 + GCH, K)
                        nidx = (c1 - c0) * P
                        nc.gpsimd.dma_gather(
                            yg[:, c0:c1, :], y1_dram[:],
                            gidx[:, t * (spt // 16) + c0 * 8:
                                 t * (spt // 16) + c1 * 8],
                            nidx, nidx, P, single_packet=False)
                    eat = wp.tile([P, K * 64], BF16, tag="eat")
                    nc.sync.dma_start(eat[:], ea_d[:, t * K * 64:(t + 1) * K * 64])
                    oh = wp.tile([P, K, W], BF16, tag="oh")
                    nc.vector.tensor_tensor(
                        out=oh[:],
                        in0=iota_b[:].to_broadcast([P, K, W]),
                        in1=dstf[:, t * K:(t + 1) * K]
                            .rearrange("p (c o) -> p c o", o=1)
                            .to_broadcast([P, K, W]),
                        op=mybir.AluOpType.is_equal)

                    ps_y = ppsA.tile([P, P], F32, tag="scat")
                    ps_a = ppsA2.tile([P, 64], F32, tag="scat_a")
                    for k in range(K):
                        wo = W * (k // KW)
                        fs = (k % KW == 0)
                        nc.tensor.matmul(ps_y[wo:wo + W, :], lhsT=oh[:, k, :],
                                         rhs=yg[:, k, :],
                                         start=fs, stop=False,
                                         skip_group_check=True)
                        nc.tensor.matmul(ps_a[wo:wo + W, :], lhsT=oh[:, k, :],
                                         rhs=eat[:, k * 64:(k + 1) * 64],
                                         start=fs, stop=(k % KW == KW - 1),
                                         skip_group_check=True)
                    # A^T tile
                    a_nm = sp.tile([P, 64], BF16, tag="a_nm")
                    nc.vector.tensor_copy(a_nm[:], ps_a[:])
                    pst = ppsB.tile([64, P], BF16, tag="trans")
                    nc.tensor.transpose(pst[:], a_nm[:], ident[:])
                    nc.vector.tensor_copy(AT[:, t * P:(t + 1) * P], pst[:])
                    for wi in range(P // W):
                        wo = W * wi
                        nc.tensor.matmul(ps_y[wo:wo + W, :],
                                         lhsT=AT[:, t * P + wo:t * P + wo + W],
                                         rhs=w["we1r"][:], start=False, stop=True,
                                         skip_group_check=True)
                    # agg -> aggT
                    agg_nm = sp.tile([P, P], BF16, tag="agg_nm")
                    nc.vector.tensor_copy(agg_nm[:], ps_y[:])
                    pst2 = ppsB.tile([P, P], BF16, tag="trans")
                    nc.tensor.transpose(pst2[:], agg_nm[:], ident[:])
                    aggT = sp.tile([P, P], BF16, tag="aggT")
                    nc.vector.tensor_copy(aggT[:], pst2[:])

                    _gru_tile(nc, (ppsD, ppsD), sp, t, aggT, xsT[:, t * P:(t + 1) * P],
                              w["wih1t"], w["whh1t"], xs[:, t * P:(t + 1) * P],
                              x1_nm[:, t * P:(t + 1) * P])
                    # x1T tile
                    h1b = sp.tile([P, P], BF16, tag="h1b")
                    nc.vector.tensor_copy(h1b[:], x1_nm[:, t * P:(t + 1) * P])
                    pst3 = ppsB.tile([P, P], BF16, tag="trans")
                    nc.tensor.transpose(pst3[:], h1b[:], ident[:])
                    nc.vector.tensor_copy(x1T[:, t * P:(t + 1) * P], pst3[:])
                    # y2 tile
                    psy2 = ppsC.tile([P, P], F32, tag="psy")
                    nc.tensor.matmul(psy2[:], lhsT=x1T[:, t * P:(t + 1) * P],
                                     rhs=w["wx2r"][:], start=True, stop=True)
                    y2b = sp.tile([P, P], BF16, tag="y2b")
                    nc.vector.tensor_copy(y2b[:], psy2[:])
                    nc.sync.dma_start(y2_loc[t * P:t * P + rows, :], y2b[0:rows, :])

                # ============================ P3: AllGather y2
                if single_core:
                    nc.sync.dma_start(y2_full[0:npc, :], y2_loc[:])
                else:
                    nc.gpsimd.collective_compute(
                        "AllGather", mybir.AluOpType.bypass,
                        replica_groups=[list(range(N_CORES))],
                        ins=[y2_loc[:].opt()], outs=[y2_full[:].opt()])

                # ============================ P4: layer 2 per dst tile
                for t in range(nt):
                    rows = min(P, npc - t * P)
                    yg = wp.tile([P, K, P], BF16, tag="yg")
                    for c0 in range(0, K, GCH):
                        c1 = min(c0 + GCH, K)
                        nidx = (c1 - c0) * P
                        nc.gpsimd.dma_gather(
                            yg[:, c0:c1, :], y2_full[:],
                            gidx[:, t * (spt // 16) + c0 * 8:
                                 t * (spt // 16) + c1 * 8],
                            nidx, nidx, P, single_packet=False)
                    oh = wp.tile([P, K, W], BF16, tag="oh")
                    nc.vector.tensor_tensor(
                        out=oh[:],
                        in0=iota_b[:].to_broadcast([P, K, W]),
                        in1=dstf[:, t * K:(t + 1) * K]
                            .rearrange("p (c o) -> p c o", o=1)
                            .to_broadcast([P, K, W]),
                        op=mybir.AluOpType.is_equal)

                    psT0 = ppsA.tile([P, W], F32, tag="scat")
                    psT1 = ppsA2.tile([P, W], F32, tag="scat_a")
                    psTs = [psT0, psT1]
                    for k in range(K):
                        psT = psTs[k // KW]
                        nc.tensor.matmul(psT[:], lhsT=yg[:, k, :], rhs=oh[:, k, :],
                                         start=(k % KW == 0), stop=False,
                                         skip_group_check=True)
                    for wi in range(P // W):
                        wo = W * wi
                        nc.tensor.matmul(psTs[wi][:], lhsT=w["we2r"][:],
                                         rhs=AT[:, t * P + wo:t * P + wo + W],
                                         start=False, stop=True,
                                         skip_group_check=True)
                    aggT2 = sp.tile([P, P], BF16, tag="aggT")
                    nc.vector.tensor_copy(aggT2[:, 0:W], psT0[:])
                    nc.vector.tensor_copy(aggT2[:, W:P], psT1[:])

                    h2 = sp.tile([P, P], F32, tag="h2")
                    _gru_tile(nc, (ppsD, ppsD), sp, t, aggT2, x1T[:, t * P:(t + 1) * P],
                              w["wih2t"], w["whh2t"], x1_nm[:, t * P:(t + 1) * P],
                              h2[:])
                    nc.sync.dma_start(out_d[t * P:t * P + rows, :], h2[0:rows, :])

    nc.compile()
    return nc


def _gru_tile(nc, psp, sp, t, mT, hT, wihT, whhT, h_nm, out_nm):
    """GRU cell for one 128-node tile, node-major output.

    mT: [128(feat), 128(node)] bf16 (aggregated message, transposed)
    hT: [128(feat), 128(node)] bf16 (hidden state, transposed)
    h_nm: [128(node), 128(feat)] f32 (hidden, node major)
    out_nm: [128(node), 128(feat)] f32 target
    """
    g_rz = psp[0].tile([P, 256], F32, tag="grz")
    g_n = psp[1].tile([P, 256], F32, tag="gn")
    # rz = gi_rz + gh_rz
    nc.tensor.matmul(g_rz[:], lhsT=mT[:], rhs=wihT[:, 0:256],
                     start=True, stop=False)
    nc.tensor.matmul(g_rz[:], lhsT=hT[:], rhs=whhT[:, 0:256],
                     start=False, stop=True)
    nc.tensor.matmul(g_n[:, 0:128], lhsT=mT[:], rhs=wihT[:, 256:384],
                     start=True, stop=True, skip_group_check=True)
    nc.tensor.matmul(g_n[:, 128:256], lhsT=hT[:], rhs=whhT[:, 256:384],
                     start=True, stop=True, skip_group_check=True)
    rz = sp.tile([P, 256], F32, tag="rz")
    nc.scalar.activation(rz[:], g_rz[:],
                         mybir.ActivationFunctionType.Sigmoid)
    tmp = sp.tile([P, P], F32, tag="gtmp")
    nc.vector.tensor_mul(tmp[:], rz[:, 0:128], g_n[:, 128:256])
    nc.vector.tensor_add(tmp[:], tmp[:], g_n[:, 0:128])
    n_t = sp.tile([P, P], F32, tag="gn")
    nc.scalar.activation(n_t[:], tmp[:], mybir.ActivationFunctionType.Tanh)
    d_t = sp.tile([P, P], F32, tag="gd")
    nc.vector.tensor_sub(d_t[:], h_nm, n_t[:])
    nc.vector.tensor_mul(d_t[:], rz[:, 128:256], d_t[:])
    nc.vector.tensor_add(out_nm, n_t[:], d_t[:])



# ---------------------------------------------------------------- entry point
_CACHE = {}


def kernel(**inputs) -> np.ndarray:
    """Full (unsharded) inputs in, full [N, 128] float32 output out.

    Shards edges by destination node across 8 NeuronCores, compiles the
    Bass kernel (cached across calls for identical graph shape), executes
    via run_bass_kernel_spmd, and concatenates + unpermutes the per-core
    node shards.
    """
    from concourse import bass_utils

    meta, in_maps = preprocess(**inputs)
    key = (meta["n_nodes"], meta["npc"], meta["nt"], meta["k_chunks"])
    nc = _CACHE.get(key)
    if nc is None:
        nc = build(meta)
        _CACHE[key] = nc
    res = bass_utils.run_bass_kernel_spmd(nc, in_maps,
                                          core_ids=list(range(N_CORES)))
    out = np.concatenate([res.results[c]["out"] for c in range(N_CORES)],
                         axis=0)
    out = out[meta["new_of_old"]]     # back to original node order
    return np.ascontiguousarray(out, dtype=np.float32)

